# revision 7
# baseline (speedup 1.0000x reference)
"""Trainium2 Bass kernel for nn_AttentionLayer (B=64, S=512, F=256), 8 cores.

Reference computation (per batch b):
    scores = x1 @ Wq + x2 @ Wk          # [S, S]
    a = softmax(tanh(scores), axis=-1)   # softmax over u
    a2 = a @ Wv                          # [S, S]
    out = a2 * x1                        # elementwise
    out = out * rsqrt(max(sum_s out^2, eps))   # l2-normalize over axis s

Strategy: pure data parallelism -- 8 batches per core, weights replicated.

v5 layout (informed by HW traces): stage A computes scores NON-transposed
([s partitions, u free], x1^T/x2^T as the stationary operand, Wq/Wk moving)
so that the softmax axis u lies on the FREE dim:
  * The softmax denominator falls out of the ACT exp pass's free-axis
    accumulator -- the 4 rowsum matmuls/batch of the transposed layout are
    eliminated (PE: 40 instead of 44 instructions per batch).
  * GpSimd normalize_recip (native Q7 op) divides exp by the accumulated
    denominator directly, producing normalized bf16 `a` -- no reciprocal,
    no partition_broadcast, no epilogue recip multiply.
  * `a` is transposed back to [u partitions, s free] for stage C with the
    hardware XBAR (dma_start_transpose, 2-byte dtype, off the PE).  The
    aT destination is laid out [P, st, ut, s'] so every transpose call
    writes a contiguous per-partition block (strided XBAR destinations
    produce wrong output on HW); stage C reads the strided [:, :, ut, :]
    view as its moving operand.
  * Stage A runs all-bf16 (same PE rate as f32r; fp8 DoubleRow is only 2x
    per MAC on HW so accuracy-safe digit splits lose).  x1 is DMA'd twice:
    bf16 for the matmul, f32 for the dtype-pure DVE epilogue (mixed-dtype
    DVE ops hit a ~2.6x slow path).
  * Epilogue per t-tile: q = y*x1 (DVE f32), sum-of-squares via DVE
    stt-accumulate, final 1/sqrt(sumsq) scale on GpSimd normalize_recip.
    Sqrt runs on ACT for batch PAIRS (halves the activation-table swaps).
  * Stage C runs with a TWO-batch skew so the exp -> normalize ->
    transpose chain of batch b completes while the PE is busy with
    A(b+1)/C(b-1); no PE work waits on it.
  * Output bf16, upcast + untransposed on host.
"""

import sys

sys.path.insert(0, "/opt/trn_rl_repo")

import numpy as np
import ml_dtypes

import concourse.bass as bass
import concourse.tile as tile
from concourse import bacc, mybir
from concourse.bass_utils import run_bass_kernel_spmd

B, S, F = 64, 512, 256
N_CORES = 8
BPC = B // N_CORES  # batches per core
P = 128
KT1 = S // P  # 4 k-tiles over t (x1/Wq contraction)
KT2 = F // P  # 2 k-tiles over f (x2/Wk contraction)
NT = S // P  # 4 tiles over s (stage A out) / t (stage C out) / u
EPS = 1e-12

F32 = mybir.dt.float32
BF16 = mybir.dt.bfloat16
AF = mybir.ActivationFunctionType
ALU = mybir.AluOpType

BFNP = ml_dtypes.bfloat16

last_results = None  # test harness introspection


def build_nc(reps=1, bpc=BPC):
    nc = bacc.Bacc(
        "TRN2", target_bir_lowering=False, debug=False, num_devices=N_CORES
    )
    # Partition-major packed tensors: [.., P, ktiles, S].
    x1a = nc.declare_dram_parameter("x1a", [bpc, P, KT1, S], BF16, isOutput=False)
    x1f = nc.declare_dram_parameter("x1f", [bpc, P, KT1, S], F32, isOutput=False)
    x2a = nc.declare_dram_parameter("x2a", [bpc, P, KT2, S], BF16, isOutput=False)
    wq = nc.declare_dram_parameter("wq", [P, KT1, S], BF16, isOutput=False)
    wk = nc.declare_dram_parameter("wk", [P, KT2, S], BF16, isOutput=False)
    wv = nc.declare_dram_parameter("wv", [P, NT, S], BF16, isOutput=False)
    out = nc.declare_dram_parameter("out", [bpc, P, NT, S], BF16, isOutput=True)

    batches = [bb for _ in range(reps) for bb in range(bpc)]

    with tile.TileContext(nc) as tc:
        with (
            tc.tile_pool(name="singles", bufs=1) as singles,
            tc.tile_pool(name="xin", bufs=1) as xin,
            tc.tile_pool(name="work", bufs=2) as work,
            tc.tile_pool(name="small", bufs=2) as small,
            tc.tile_pool(name="outp", bufs=2) as outp,
            tc.tile_pool(name="psA", bufs=2, space="PSUM") as psA,
            tc.tile_pool(name="psY", bufs=4, space="PSUM") as psY,
        ):
            # Startup DMAs in consumption order (first s-tile needs x1.kt +
            # wq.kt pairs, then x2 + wk), spread across queues.
            b0 = batches[0]
            x1_first = xin.tile([P, KT1, S], BF16, tag="x1", bufs=3)
            wq_t = singles.tile([P, KT1, S], BF16, tag="wq")
            wk_t = singles.tile([P, KT2, S], BF16, tag="wk")
            x2_first = xin.tile([P, KT2, S], BF16, tag="x2", bufs=3)
            wv_t = singles.tile([P, NT, S], BF16, tag="wv")
            wq_engs = [nc.scalar, nc.scalar, nc.gpsimd, nc.gpsimd]
            for kt in range(KT1):
                nc.sync.dma_start(
                    out=x1_first[:, kt : kt + 1, :],
                    in_=x1a.ap()[b0, :, kt : kt + 1, :],
                )
                wq_engs[kt].dma_start(
                    out=wq_t[:, kt : kt + 1, :], in_=wq.ap()[:, kt : kt + 1, :]
                )
            nc.scalar.dma_start(out=wk_t, in_=wk.ap())
            nc.sync.dma_start(out=x2_first, in_=x2a.ap()[b0])
            nc.gpsimd.dma_start(out=wv_t, in_=wv.ap())
            x1f_first = xin.tile([P, KT1, S], F32, tag="x1f", bufs=4)
            nc.gpsimd.dma_start(out=x1f_first, in_=x1f.ap()[b0])

            eps_t = singles.tile([P, 1], F32)
            nc.vector.memset(eps_t, EPS)

            def stage_a(b, x1_sb, x2_sb, mid_cb=None):
                """scores[s,u] matmuls (bf16, x-tiles stationary / weights
                moving) into s-tile-pair PSUM, tanh per pair, exp per s-tile
                with the free-axis accumulator producing the softmax
                denominator.  mid_cb is emitted between halves."""
                ez = work.tile([P, NT, S], F32, tag="ez", bufs=2)
                dn = small.tile([P, NT], F32, tag="dn", bufs=4)
                for half in range(NT // 2):
                    sc = psA.tile([P, 2, S], F32, tag="scores")
                    for j in range(2):
                        st = half * 2 + j
                        ss = slice(st * P, (st + 1) * P)
                        for kt in range(KT1):
                            nc.tensor.matmul(
                                sc[:, j, :],
                                x1_sb[:, kt, ss],
                                wq_t[:, kt, :],
                                start=(kt == 0),
                                stop=False,
                            )
                        for kt in range(KT2):
                            nc.tensor.matmul(
                                sc[:, j, :],
                                x2_sb[:, kt, ss],
                                wk_t[:, kt, :],
                                start=False,
                                stop=(kt == KT2 - 1),
                            )
                    tanh_t = work.tile([P, 2, S], F32, tag="tanh")
                    nc.scalar.activation(out=tanh_t, in_=sc, func=AF.Tanh)
                    for j in range(2):
                        st = half * 2 + j
                        nc.scalar.activation(
                            out=ez[:, st, :],
                            in_=tanh_t[:, j, :],
                            func=AF.Exp,
                            accum_out=dn[:, st : st + 1],
                        )
                    if half == 0 and mid_cb is not None:
                        mid_cb()
                return ez, dn

            def stage_b(b, ez, dn):
                """softmax normalize on GpSimd (divide by the accumulated
                denominator), then XBAR-transpose a -> aT for stage C."""
                a_t = work.tile([P, NT, S], BF16, tag="a", bufs=2)
                for st in range(NT):
                    nc.gpsimd.normalize_recip(
                        out_ap=a_t[:, st, :],
                        in_ap=ez[:, st, :],
                        denom_ap=dn[:, st : st + 1],
                    )
                aT = work.tile([P, NT, NT, P], BF16, tag="aT", bufs=3)
                tr_engs = [nc.sync, nc.scalar, nc.sync, nc.scalar]
                for st in range(NT):
                    tr_engs[st].dma_start_transpose(
                        out=aT[:, st, :, :], in_=a_t[:, st, :]
                    )
                return aT

            def stage_c(b, x1f_sb, aT):
                """Y matmuls on aT; epilogue q = y*x1 (f32 DVE) and
                sum-of-squares via DVE stt-accumulate."""
                q_sb = outp.tile([P, NT, S], F32, tag="q", bufs=3)
                sumsq = small.tile([P, NT], F32, tag="sumsq", bufs=4)
                for tt in range(NT):
                    y = psY.tile([P, S], F32, tag="y")
                    for ut in range(NT):
                        nc.tensor.matmul(
                            y,
                            wv_t[:, ut, tt * P : (tt + 1) * P],
                            aT[:, :, ut, :],
                            start=(ut == 0),
                            stop=(ut == NT - 1),
                        )
                    q_t = q_sb[:, tt, :]
                    nc.vector.tensor_tensor(
                        out=q_t, in0=y, in1=x1f_sb[:, tt, :], op=ALU.mult
                    )
                    scr = small.tile([P, S], F32, tag="scr")
                    nc.vector.scalar_tensor_tensor(
                        out=scr,
                        in0=q_t,
                        scalar=1.0,
                        in1=q_t,
                        op0=ALU.mult,
                        op1=ALU.mult,
                        accum_out=sumsq[:, tt : tt + 1],
                    )
                return q_sb, sumsq

            def stage_fin(b, q_sb, sumsq):
                """sqrt (ACT, emitted adjacently for pairs of batches to halve
                activation-table swaps), GpSimd normalize, store bf16."""
                rsq = small.tile([P, NT], F32, tag="rsq", bufs=4)
                nc.scalar.activation(out=rsq, in_=sumsq, func=AF.Sqrt, bias=eps_t)
                ob = outp.tile([P, NT, S], BF16, tag="ob")
                for tt in range(NT):
                    nc.gpsimd.normalize_recip(
                        out_ap=ob[:, tt, :],
                        in_ap=q_sb[:, tt, :],
                        denom_ap=rsq[:, tt : tt + 1],
                    )
                nc.scalar.dma_start(out=out.ap()[b], in_=ob)

            def stage_fin_last(b, q_sb, sumsq):
                """Drain-batch finalize: normalizes split Pool/DVE, output
                stored in two half-DMAs so only ~0.25MB is tail-exposed."""
                rsq = small.tile([P, NT], F32, tag="rsq", bufs=4)
                nc.scalar.activation(out=rsq, in_=sumsq, func=AF.Sqrt, bias=eps_t)
                vv = small.tile([P, NT], F32, tag="vv")
                nc.vector.reciprocal_approx_fast(out=vv, in_=rsq)
                ob = outp.tile([P, NT, S], BF16, tag="ob")
                for tt in range(NT):
                    if tt % 2 == 0:
                        nc.gpsimd.normalize_recip(
                            out_ap=ob[:, tt, :],
                            in_ap=q_sb[:, tt, :],
                            denom_ap=rsq[:, tt : tt + 1],
                        )
                    else:
                        nc.vector.tensor_scalar_mul(
                            ob[:, tt, :], q_sb[:, tt, :], vv[:, tt : tt + 1]
                        )
                    if tt == 1:
                        nc.scalar.dma_start(
                            out=out.ap()[b, :, 0:2, :], in_=ob[:, 0:2, :]
                        )
                nc.scalar.dma_start(out=out.ap()[b, :, 2:4, :], in_=ob[:, 2:4, :])

            def dma_x(b):
                t1 = xin.tile([P, KT1, S], BF16, tag="x1", bufs=3)
                nc.sync.dma_start(out=t1, in_=x1a.ap()[b])
                t2 = xin.tile([P, KT2, S], BF16, tag="x2", bufs=3)
                nc.sync.dma_start(out=t2, in_=x2a.ap()[b])
                tf = xin.tile([P, KT1, S], F32, tag="x1f", bufs=4)
                nc.gpsimd.dma_start(out=tf, in_=x1f.ap()[b])
                return t1, t2, tf

            # Two-batch skew: A(b) || norm/transpose(b-1) || C(b-2).
            norm_q = None  # (b, ez, dn) awaiting stage B at mid-A(next)
            c_q = None  # (b, x1f_sb, aT) awaiting stage C
            fins = []  # (b, q_sb, sumsq) awaiting finalize, in pairs
            x1_cur, x2_cur, x1f_cur = x1_first, x2_first, x1f_first
            for i, b in enumerate(batches):
                if i + 1 < len(batches):
                    nxt = dma_x(batches[i + 1])
                else:
                    nxt = (None, None, None)
                prev_n, hold = norm_q, {}

                def mid_cb():
                    hold["aT"] = stage_b(prev_n[0], prev_n[1], prev_n[2])

                ez, dn = stage_a(
                    b, x1_cur, x2_cur, mid_cb if prev_n is not None else None
                )
                if c_q is not None:
                    fins.append((c_q[0],) + stage_c(*c_q))
                    if len(fins) == 2:
                        for f in fins:
                            stage_fin(*f)
                        fins = []
                if prev_n is not None:
                    c_q = (prev_n[0], prev_n[3], hold["aT"])
                norm_q = (b, ez, dn, x1f_cur)
                x1_cur, x2_cur, x1f_cur = nxt
            # drain: B(7); C(6); flush fin pair; C(7); finalize 7 last.
            aT7 = stage_b(norm_q[0], norm_q[1], norm_q[2])
            fins.append((c_q[0],) + stage_c(*c_q))
            for f in fins:
                stage_fin(*f)
            last_c = stage_c(norm_q[0], norm_q[3], aT7)
            stage_fin_last(norm_q[0], *last_c)

    nc.compile()
    return nc


def _pack_pmajor(a, nchunks):
    """[.., nchunks*P, S] -> [.., P, nchunks, S] partition-major contiguous."""
    lead = a.shape[:-2]
    a = a.reshape(lead + (nchunks, P, S))
    perm = tuple(range(len(lead))) + (len(lead) + 1, len(lead), len(lead) + 2)
    return np.ascontiguousarray(a.transpose(perm))


_nc_cache = None


def kernel(x1, x2, W_query, W_key, W_value, _trace=False):
    global _nc_cache, last_results
    x1t = np.asarray(x1, dtype=np.float32).transpose(0, 2, 1)  # [B, t, s]
    x1a = _pack_pmajor(x1t.astype(BFNP), KT1)
    x1f = _pack_pmajor(x1t, KT1)
    x2a = _pack_pmajor(
        np.asarray(x2, dtype=np.float32).transpose(0, 2, 1).astype(BFNP), KT2
    )
    wq = _pack_pmajor(np.asarray(W_query, dtype=np.float32).astype(BFNP), KT1)
    wk = _pack_pmajor(np.asarray(W_key, dtype=np.float32).astype(BFNP), KT2)
    wv = _pack_pmajor(np.asarray(W_value, dtype=np.float32).astype(BFNP), NT)

    if _nc_cache is None:
        _nc_cache = build_nc()
    nc = _nc_cache

    in_maps = []
    for c in range(N_CORES):
        sl = slice(c * BPC, (c + 1) * BPC)
        in_maps.append(
            {
                "x1a": x1a[sl],
                "x1f": x1f[sl],
                "x2a": x2a[sl],
                "wq": wq,
                "wk": wk,
                "wv": wv,
            }
        )
    res = run_bass_kernel_spmd(
        nc, in_maps, core_ids=list(range(N_CORES)), trace=_trace
    )
    last_results = res
    # out: [bpc, P, NT, S] bf16 -> outT [B, S, S] -> untranspose
    outs = [np.asarray(res.results[c]["out"]) for c in range(N_CORES)]
    outT = np.concatenate(outs, axis=0).astype(np.float32)
    outT = outT.transpose(0, 2, 1, 3).reshape(B, S, S)
    return np.ascontiguousarray(outT.transpose(0, 2, 1))


# revision 10
# speedup vs baseline: 1.2056x; 1.2056x over previous
"""Trainium2 Bass kernel for nn_AttentionLayer (B=64, S=512, F=256), 8 cores.

Reference computation (per batch b):
    scores = x1 @ Wq + x2 @ Wk          # [S, S]
    a = softmax(tanh(scores), axis=-1)   # softmax over u
    a2 = a @ Wv                          # [S, S]
    out = a2 * x1                        # elementwise
    out = out * rsqrt(max(sum_s out^2, eps))   # l2-normalize over axis s

Strategy: pure data parallelism -- 8 batches per core, weights replicated.
Everything is computed in a TRANSPOSED layout ([t-or-u partitions, s free]).

v6 design notes (informed by HW traces):
  * All matmuls in bf16 (same PE rate as f32r on HW; fp8 DoubleRow is only
    2x per MAC so accuracy-safe digit splits lose; bf16 stationary loads
    are also slightly faster than f32r).  x1 is DMA'd twice: bf16 for
    stage A, f32 (lazily, off the critical path) for the dtype-pure DVE
    epilogue -- mixed-dtype DVE ops hit a ~2.6x slow path.
  * Stage C consumes the UNNORMALIZED expz; 1/denominator folds into the
    epilogue, so no PE work waits on the rowsum->recip->broadcast chain.
  * Epilogue per t-tile: q = y*x1 and w = q*recip_bc (f32 tensor_tensor on
    DVE); sum-of-squares split between ACT Square+accum (same activation
    table as exp/tanh) and DVE stt to balance load; final per-row
    1/sqrt(sumsq) scale on GpSimd normalize_recip (native Q7 op; gpsimd
    tensor_scalar is a ~7.6us/tile software trap -- never use).
  * Sqrt runs on ACT in batches of FOUR, and the drain shares a single
    sqrt-table epoch (the last batch's squares run on DVE), cutting
    activation-table loads from 12 to 3 (~1.3us each).
  * The drain batch's epilogue is split across DVE/Pool/ACT and its output
    stored in two half-DMAs to shorten the serial tail.
  * All DRAM tensors partition-major; output bf16, upcast on host.
"""

import sys

sys.path.insert(0, "/opt/trn_rl_repo")

import numpy as np
import ml_dtypes

import concourse.bass as bass
import concourse.tile as tile
from concourse import bacc, mybir
from concourse.bass_utils import run_bass_kernel_spmd

B, S, F = 64, 512, 256
N_CORES = 8
BPC = B // N_CORES  # batches per core
P = 128
KT1 = S // P  # 4 k-tiles over t (x1/Wq contraction)
KT2 = F // P  # 2 k-tiles over f (x2/Wk contraction)
NT = S // P  # 4 m-tiles over u (stage A) / t (stage C)
EPS = 1e-12
FIN_GROUP = 4  # batches per sqrt-table epoch

F32 = mybir.dt.float32
BF16 = mybir.dt.bfloat16
AF = mybir.ActivationFunctionType
ALU = mybir.AluOpType

BFNP = ml_dtypes.bfloat16

last_results = None  # test harness introspection


def build_nc(reps=1, bpc=BPC):
    nc = bacc.Bacc(
        "TRN2", target_bir_lowering=False, debug=False, num_devices=N_CORES
    )
    # Partition-major packed tensors: [.., P, ktiles, S].
    x1a = nc.declare_dram_parameter("x1a", [bpc, P, KT1, S], BF16, isOutput=False)
    x1f = nc.declare_dram_parameter("x1f", [bpc, P, KT1, S], F32, isOutput=False)
    x2a = nc.declare_dram_parameter("x2a", [bpc, P, KT2, S], BF16, isOutput=False)
    wq = nc.declare_dram_parameter("wq", [P, KT1, S], BF16, isOutput=False)
    wk = nc.declare_dram_parameter("wk", [P, KT2, S], BF16, isOutput=False)
    wv = nc.declare_dram_parameter("wv", [P, NT, S], BF16, isOutput=False)
    out = nc.declare_dram_parameter("out", [bpc, P, NT, S], BF16, isOutput=True)

    batches = [bb for _ in range(reps) for bb in range(bpc)]

    with tile.TileContext(nc) as tc:
        with (
            tc.tile_pool(name="singles", bufs=1) as singles,
            tc.tile_pool(name="xin", bufs=1) as xin,
            tc.tile_pool(name="work", bufs=2) as work,
            tc.tile_pool(name="small", bufs=2) as small,
            tc.tile_pool(name="outp", bufs=2) as outp,
            tc.tile_pool(name="psA", bufs=2, space="PSUM") as psA,
            tc.tile_pool(name="psY", bufs=3, space="PSUM") as psY,
            tc.tile_pool(name="psR", bufs=1, space="PSUM") as psR,
        ):
            # Startup: only x1a+wq (1MB bf16) gate the first matmuls; x1f
            # (epilogue copy) arrives lazily on the gpsimd queue.
            b0 = batches[0]
            x1_first = xin.tile([P, KT1, S], BF16, tag="x1", bufs=3)
            nc.sync.dma_start(out=x1_first, in_=x1a.ap()[b0])
            wq_t = singles.tile([P, KT1, S], BF16, tag="wq")
            nc.scalar.dma_start(out=wq_t[:, 0:2, :], in_=wq.ap()[:, 0:2, :])
            nc.scalar.dma_start(out=wq_t[:, 2:4, :], in_=wq.ap()[:, 2:4, :])
            wk_t = singles.tile([P, KT2, S], BF16, tag="wk")
            nc.gpsimd.dma_start(out=wk_t, in_=wk.ap())
            x2_first = xin.tile([P, KT2, S], BF16, tag="x2", bufs=3)
            nc.sync.dma_start(out=x2_first, in_=x2a.ap()[b0])
            wv_t = singles.tile([P, NT, S], BF16, tag="wv")
            nc.gpsimd.dma_start(out=wv_t, in_=wv.ap())
            x1f_first = xin.tile([P, KT1, S], F32, tag="x1f", bufs=3)
            nc.gpsimd.dma_start(out=x1f_first, in_=x1f.ap()[b0])

            ones_col = singles.tile([P, 1], BF16)
            nc.vector.memset(ones_col, 1.0)
            eps_t = singles.tile([P, 1], F32)
            nc.vector.memset(eps_t, EPS)

            def stage_a(b, x1_sb, x2_sb, mid_cb=None):
                """scores matmuls (bf16) in u-tile pairs sharing one 2-bank
                PSUM tile, tanh+exp over pairs.  mid_cb is emitted between
                the halves so the previous batch's rowsum overlaps."""
                expz = work.tile([P, NT, S], BF16, tag="expz", bufs=3)
                for half in range(NT // 2):
                    sc = psA.tile([P, 2, S], F32, tag="scores")
                    for j in range(2):
                        ut = half * 2 + j
                        us = slice(ut * P, (ut + 1) * P)
                        for kt in range(KT1):
                            nc.tensor.matmul(
                                sc[:, j, :],
                                wq_t[:, kt, us],
                                x1_sb[:, kt, :],
                                start=(kt == 0),
                                stop=False,
                            )
                        for kt in range(KT2):
                            nc.tensor.matmul(
                                sc[:, j, :],
                                wk_t[:, kt, us],
                                x2_sb[:, kt, :],
                                start=False,
                                stop=(kt == KT2 - 1),
                            )
                    tanh_t = work.tile([P, 2, S], F32, tag="tanh")
                    nc.scalar.activation(out=tanh_t, in_=sc, func=AF.Tanh)
                    nc.scalar.activation(
                        out=expz[:, half * 2 : half * 2 + 2, :],
                        in_=tanh_t,
                        func=AF.Exp,
                    )
                    if half == 0 and mid_cb is not None:
                        mid_cb()
                return expz

            def stage_b(b, expz):
                """softmax denominator: rowsum matmul -> recip -> f32 bcast."""
                rs = psR.tile([1, S], F32, tag="rowsum")
                for ut in range(NT):
                    nc.tensor.matmul(
                        rs,
                        ones_col,
                        expz[:, ut, :],
                        start=(ut == 0),
                        stop=(ut == NT - 1),
                    )
                recip_f = small.tile([1, S], F32, tag="recipf")
                nc.vector.reciprocal_approx_fast(out=recip_f, in_=rs)
                rbc = small.tile([P, S], F32, tag="rbc")
                nc.gpsimd.partition_broadcast(rbc, recip_f)
                return rbc

            def stage_c(b, x1f_sb, expz, rbc):
                """Y matmuls on raw expz; epilogue q=y*x1 -> w=q*rbc (f32 on
                DVE); sum-of-squares split between ACT Square+accum (Square
                lives in both activation table sets, so it never forces a
                swap) and DVE stt to balance engine load."""
                w_sb = outp.tile([P, NT, S], F32, tag="w", bufs=FIN_GROUP + 2)
                sumsq = small.tile([P, NT], F32, tag="sumsq", bufs=FIN_GROUP + 2)
                for tt in range(NT):
                    y = psY.tile([P, S], F32, tag="y")
                    for ut in range(NT):
                        nc.tensor.matmul(
                            y,
                            wv_t[:, ut, tt * P : (tt + 1) * P],
                            expz[:, ut, :],
                            start=(ut == 0),
                            stop=(ut == NT - 1),
                        )
                    q_t = small.tile([P, S], F32, tag="q")
                    w_t = w_sb[:, tt, :]
                    nc.vector.tensor_tensor(
                        out=q_t, in0=y, in1=x1f_sb[:, tt, :], op=ALU.mult
                    )
                    nc.vector.tensor_tensor(out=w_t, in0=q_t, in1=rbc, op=ALU.mult)
                    if tt >= 2:
                        scr = small.tile([P, S], F32, tag="scr")
                        nc.vector.scalar_tensor_tensor(
                            out=scr,
                            in0=w_t,
                            scalar=1.0,
                            in1=w_t,
                            op0=ALU.mult,
                            op1=ALU.mult,
                            accum_out=sumsq[:, tt : tt + 1],
                        )
                    else:
                        scr = small.tile([P, S], BF16, tag="scrb")
                        nc.scalar.activation(
                            out=scr,
                            in_=w_t,
                            func=AF.Square,
                            accum_out=sumsq[:, tt : tt + 1],
                        )
                return w_sb, sumsq

            def stage_fin(b, w_sb, sumsq):
                """sqrt on ACT (finalizes emitted in groups so several
                batches share one sqrt-table epoch), GpSimd normalize."""
                rsq = small.tile([P, NT], F32, tag="rsq", bufs=FIN_GROUP + 2)
                nc.scalar.activation(out=rsq, in_=sumsq, func=AF.Sqrt, bias=eps_t)
                ob = outp.tile([P, NT, S], BF16, tag="ob")
                for tt in range(NT):
                    nc.gpsimd.normalize_recip(
                        out_ap=ob[:, tt, :],
                        in_ap=w_sb[:, tt, :],
                        denom_ap=rsq[:, tt : tt + 1],
                    )
                nc.scalar.dma_start(out=out.ap()[b], in_=ob)

            def stage_fin_last(b, w_sb, sumsq):
                """Drain finalize: normalizes split Pool/DVE, two half-DMAs."""
                rsq = small.tile([P, NT], F32, tag="rsq", bufs=FIN_GROUP + 2)
                nc.scalar.activation(out=rsq, in_=sumsq, func=AF.Sqrt, bias=eps_t)
                vv = small.tile([P, NT], F32, tag="vv")
                nc.vector.reciprocal_approx_fast(out=vv, in_=rsq)
                ob = outp.tile([P, NT, S], BF16, tag="ob")
                for tt in range(NT):
                    if tt % 2 == 0:
                        nc.gpsimd.normalize_recip(
                            out_ap=ob[:, tt, :],
                            in_ap=w_sb[:, tt, :],
                            denom_ap=rsq[:, tt : tt + 1],
                        )
                    else:
                        nc.vector.tensor_scalar_mul(
                            ob[:, tt, :], w_sb[:, tt, :], vv[:, tt : tt + 1]
                        )
                    if tt == 1:
                        nc.scalar.dma_start(
                            out=out.ap()[b, :, 0:2, :], in_=ob[:, 0:2, :]
                        )
                nc.scalar.dma_start(out=out.ap()[b, :, 2:4, :], in_=ob[:, 2:4, :])

            def dma_x(b):
                t1 = xin.tile([P, KT1, S], BF16, tag="x1", bufs=3)
                nc.sync.dma_start(out=t1, in_=x1a.ap()[b])
                t2 = xin.tile([P, KT2, S], BF16, tag="x2", bufs=3)
                nc.sync.dma_start(out=t2, in_=x2a.ap()[b])
                tf = xin.tile([P, KT1, S], F32, tag="x1f", bufs=3)
                nc.gpsimd.dma_start(out=tf, in_=x1f.ap()[b])
                return t1, t2, tf

            pending = None  # (b, x1f_sb, expz) awaiting stages B+C
            fins = []  # (b, w_sb, sumsq) awaiting finalize, in quads
            x1_cur, x2_cur, x1f_cur = x1_first, x2_first, x1f_first
            for i, b in enumerate(batches):
                if i + 1 < len(batches):
                    nxt = dma_x(batches[i + 1])
                else:
                    nxt = (None, None, None)
                prev = pending
                hold = {}

                def mid_cb():
                    hold["rbc"] = stage_b(prev[0], prev[2])

                expz = stage_a(
                    b, x1_cur, x2_cur, mid_cb if prev is not None else None
                )
                if prev is not None:
                    fins.append(
                        (prev[0],) + stage_c(prev[0], prev[1], prev[2], hold["rbc"])
                    )
                    if len(fins) == FIN_GROUP:
                        for f in fins:
                            stage_fin(*f)
                        fins = []
                pending = (b, x1f_cur, expz)
                x1_cur, x2_cur, x1f_cur = nxt
            # drain: flush pending finalizes (their sqrt-table epoch is
            # shared with the last batch: its squares run off-ACT), then the
            # last batch's stage C + split finalize.
            rbc_last = stage_b(pending[0], pending[2])
            last_c = stage_c(pending[0], pending[1], pending[2], rbc_last)
            for f in fins:
                stage_fin(*f)
            stage_fin_last(pending[0], *last_c)

    nc.compile()
    return nc


def _pack_pmajor(a, nchunks):
    """[.., nchunks*P, S] -> [.., P, nchunks, S] partition-major contiguous."""
    lead = a.shape[:-2]
    a = a.reshape(lead + (nchunks, P, S))
    perm = tuple(range(len(lead))) + (len(lead) + 1, len(lead), len(lead) + 2)
    return np.ascontiguousarray(a.transpose(perm))


_nc_cache = None


def kernel(x1, x2, W_query, W_key, W_value, _trace=False):
    global _nc_cache, last_results
    x1t = np.asarray(x1, dtype=np.float32).transpose(0, 2, 1)  # [B, t, s]
    x1a = _pack_pmajor(x1t.astype(BFNP), KT1)
    x1f = _pack_pmajor(x1t, KT1)
    x2a = _pack_pmajor(
        np.asarray(x2, dtype=np.float32).transpose(0, 2, 1).astype(BFNP), KT2
    )
    wq = _pack_pmajor(np.asarray(W_query, dtype=np.float32).astype(BFNP), KT1)
    wk = _pack_pmajor(np.asarray(W_key, dtype=np.float32).astype(BFNP), KT2)
    wv = _pack_pmajor(np.asarray(W_value, dtype=np.float32).astype(BFNP), NT)

    if _nc_cache is None:
        _nc_cache = build_nc()
    nc = _nc_cache

    in_maps = []
    for c in range(N_CORES):
        sl = slice(c * BPC, (c + 1) * BPC)
        in_maps.append(
            {
                "x1a": x1a[sl],
                "x1f": x1f[sl],
                "x2a": x2a[sl],
                "wq": wq,
                "wk": wk,
                "wv": wv,
            }
        )
    res = run_bass_kernel_spmd(
        nc, in_maps, core_ids=list(range(N_CORES)), trace=_trace
    )
    last_results = res
    # out: [bpc, P, NT, S] bf16 -> outT [B, S, S] -> untranspose
    outs = [np.asarray(res.results[c]["out"]) for c in range(N_CORES)]
    outT = np.concatenate(outs, axis=0).astype(np.float32)
    outT = outT.transpose(0, 2, 1, 3).reshape(B, S, S)
    return np.ascontiguousarray(outT.transpose(0, 2, 1))


# revision 12
# speedup vs baseline: 1.3496x; 1.1194x over previous
"""Trainium2 Bass kernel for nn_AttentionLayer (B=64, S=512, F=256), 8 cores.

Reference computation (per batch b):
    scores = x1 @ Wq + x2 @ Wk          # [S, S]
    a = softmax(tanh(scores), axis=-1)   # softmax over u
    a2 = a @ Wv                          # [S, S]
    out = a2 * x1                        # elementwise
    out = out * rsqrt(max(sum_s out^2, eps))   # l2-normalize over axis s

Strategy: pure data parallelism -- 8 batches per core, weights replicated.
Everything is computed in a TRANSPOSED layout ([t-or-u partitions, s free]).

v7 design notes (informed by HW traces):
  * x1 is DMA'd ONCE as float32r: stage A streams it as the moving matmul
    operand (1 cycle/row -- same PE rate as bf16), and the epilogue
    bitcasts the same SBUF bytes to f32.  A second bf16 copy of x1 was
    measured to stretch every engine ~20% via SBUF port contention.
  * All weights + x2 are bf16 (halves their DMA; bf16 stationary loads
    avoid the f32r weight-load stretch on stage-A instructions).
  * fp8 DoubleRow measures only 2x per MAC on this HW, so accuracy-safe
    hi+lo digit splits lose to f32r/bf16 -- not used.
  * Stage C consumes the UNNORMALIZED expz; 1/denominator folds into the
    epilogue, so no PE work waits on the rowsum->recip->broadcast chain.
  * Epilogue per t-tile: q = y*x1 and w = q*recip_bc as dtype-pure f32
    tensor_tensor on DVE (mixed-dtype DVE ops hit a ~2.6x slow path);
    sum-of-squares split between ACT Square+accum (Square lives in both
    activation-table sets -- never forces a swap) and DVE stt; final
    per-row 1/sqrt scale on GpSimd normalize_recip (native Q7 op; gpsimd
    tensor_scalar is a ~7.6us/tile software trap -- never use).
  * Sqrt on ACT for batch PAIRS; the drain shares its sqrt-table epoch.
  * Drain finalize: normalizes split Pool/DVE, output in two half-DMAs.
  * All DRAM tensors partition-major; output bf16, upcast on host.
"""

import sys

sys.path.insert(0, "/opt/trn_rl_repo")

import numpy as np
import ml_dtypes

import concourse.bass as bass
import concourse.tile as tile
from concourse import bacc, mybir
from concourse.bass_utils import run_bass_kernel_spmd

B, S, F = 64, 512, 256
N_CORES = 8
BPC = B // N_CORES  # batches per core
P = 128
KT1 = S // P  # 4 k-tiles over t (x1/Wq contraction)
KT2 = F // P  # 2 k-tiles over f (x2/Wk contraction)
NT = S // P  # 4 m-tiles over u (stage A) / t (stage C)
EPS = 1e-12

F32 = mybir.dt.float32
F32R = mybir.dt.float32r
BF16 = mybir.dt.bfloat16
AF = mybir.ActivationFunctionType
ALU = mybir.AluOpType

BFNP = ml_dtypes.bfloat16

last_results = None  # test harness introspection


def build_nc(reps=1, bpc=BPC):
    nc = bacc.Bacc(
        "TRN2", target_bir_lowering=False, debug=False, num_devices=N_CORES
    )
    # Partition-major packed tensors: [.., P, ktiles, S].
    x1t = nc.declare_dram_parameter("x1t", [bpc, P, KT1, S], F32R, isOutput=False)
    x2t = nc.declare_dram_parameter("x2t", [bpc, P, KT2, S], BF16, isOutput=False)
    wq = nc.declare_dram_parameter("wq", [P, KT1, S], F32R, isOutput=False)
    wk = nc.declare_dram_parameter("wk", [P, KT2, S], BF16, isOutput=False)
    wv = nc.declare_dram_parameter("wv", [P, NT, S], BF16, isOutput=False)
    out = nc.declare_dram_parameter("out", [bpc, P, NT, S], BF16, isOutput=True)

    batches = [bb for _ in range(reps) for bb in range(bpc)]

    with tile.TileContext(nc) as tc:
        with (
            tc.tile_pool(name="singles", bufs=1) as singles,
            tc.tile_pool(name="xin", bufs=1) as xin,
            tc.tile_pool(name="work", bufs=2) as work,
            tc.tile_pool(name="small", bufs=2) as small,
            tc.tile_pool(name="outp", bufs=2) as outp,
            tc.tile_pool(name="psA", bufs=2, space="PSUM") as psA,
            tc.tile_pool(name="psY", bufs=3, space="PSUM") as psY,
            tc.tile_pool(name="psR", bufs=1, space="PSUM") as psR,
        ):
            # Startup DMAs in consumption order: the first u-tile needs
            # wq + x1.kt pairs in sequence, then wk + x2.
            b0 = batches[0]
            x1_first = xin.tile([P, KT1, S], F32R, tag="x1", bufs=3)
            wq_t = singles.tile([P, KT1, S], F32R, tag="wq")
            nc.scalar.dma_start(out=wq_t[:, 0:2, :], in_=wq.ap()[:, 0:2, :])
            nc.scalar.dma_start(out=wq_t[:, 2:4, :], in_=wq.ap()[:, 2:4, :])
            nc.sync.dma_start(out=x1_first[:, 0:2, :], in_=x1t.ap()[b0, :, 0:2, :])
            wk_t = singles.tile([P, KT2, S], BF16, tag="wk")
            nc.gpsimd.dma_start(out=wk_t, in_=wk.ap())
            nc.sync.dma_start(out=x1_first[:, 2:4, :], in_=x1t.ap()[b0, :, 2:4, :])
            x2_first = xin.tile([P, KT2, S], BF16, tag="x2", bufs=3)
            nc.sync.dma_start(out=x2_first, in_=x2t.ap()[b0])
            wv_t = singles.tile([P, NT, S], BF16, tag="wv")
            nc.gpsimd.dma_start(out=wv_t, in_=wv.ap())

            ones_col = singles.tile([P, 1], BF16)
            nc.vector.memset(ones_col, 1.0)
            eps_t = singles.tile([P, 1], F32)
            nc.vector.memset(eps_t, EPS)

            def stage_a(b, x1_sb, x2_sb, mid_cb=None):
                """scores matmuls in u-tile pairs sharing one 2-bank PSUM
                tile, tanh+exp over pairs.  mid_cb (if set) is emitted
                between the two pair-halves so the previous batch's rowsum
                overlaps this batch's remaining matmuls."""
                expz = work.tile([P, NT, S], BF16, tag="expz", bufs=3)
                for half in range(NT // 2):
                    sc = psA.tile([P, 2, S], F32, tag="scores")
                    for j in range(2):
                        ut = half * 2 + j
                        us = slice(ut * P, (ut + 1) * P)
                        for kt in range(KT1):
                            nc.tensor.matmul(
                                sc[:, j, :],
                                wq_t[:, kt, us],
                                x1_sb[:, kt, :],
                                start=(kt == 0),
                                stop=False,
                            )
                        for kt in range(KT2):
                            nc.tensor.matmul(
                                sc[:, j, :],
                                wk_t[:, kt, us],
                                x2_sb[:, kt, :],
                                start=False,
                                stop=(kt == KT2 - 1),
                            )
                    tanh_t = work.tile([P, 2, S], F32, tag="tanh")
                    nc.scalar.activation(out=tanh_t, in_=sc, func=AF.Tanh)
                    nc.scalar.activation(
                        out=expz[:, half * 2 : half * 2 + 2, :],
                        in_=tanh_t,
                        func=AF.Exp,
                    )
                    if half == 0 and mid_cb is not None:
                        mid_cb()
                return expz

            def stage_b(b, expz):
                """softmax denominator: rowsum matmul -> recip -> f32 bcast."""
                rs = psR.tile([1, S], F32, tag="rowsum")
                for ut in range(NT):
                    nc.tensor.matmul(
                        rs,
                        ones_col,
                        expz[:, ut, :],
                        start=(ut == 0),
                        stop=(ut == NT - 1),
                    )
                recip_f = small.tile([1, S], F32, tag="recipf")
                nc.vector.reciprocal_approx_fast(out=recip_f, in_=rs)
                rbc = small.tile([P, S], F32, tag="rbc")
                nc.gpsimd.partition_broadcast(rbc, recip_f)
                return rbc

            def stage_c(b, x1_sb, expz, rbc):
                """Y matmuls on raw expz; epilogue q=y*x1 -> w=q*rbc (f32 on
                DVE); sum-of-squares split between ACT Square+accum and DVE
                stt to balance engine load."""
                w_sb = outp.tile([P, NT, S], F32, tag="w", bufs=3)
                sumsq = small.tile([P, NT], F32, tag="sumsq", bufs=4)
                for tt in range(NT):
                    y = psY.tile([P, S], F32, tag="y")
                    for ut in range(NT):
                        nc.tensor.matmul(
                            y,
                            wv_t[:, ut, tt * P : (tt + 1) * P],
                            expz[:, ut, :],
                            start=(ut == 0),
                            stop=(ut == NT - 1),
                        )
                    q_t = small.tile([P, S], F32, tag="q")
                    w_t = w_sb[:, tt, :]
                    nc.vector.tensor_tensor(
                        out=q_t, in0=y, in1=x1_sb[:, tt, :].bitcast(F32), op=ALU.mult
                    )
                    nc.vector.tensor_tensor(out=w_t, in0=q_t, in1=rbc, op=ALU.mult)
                    if tt >= 2:
                        scr = small.tile([P, S], F32, tag="scr")
                        nc.vector.scalar_tensor_tensor(
                            out=scr,
                            in0=w_t,
                            scalar=1.0,
                            in1=w_t,
                            op0=ALU.mult,
                            op1=ALU.mult,
                            accum_out=sumsq[:, tt : tt + 1],
                        )
                    else:
                        scr = small.tile([P, S], BF16, tag="scrb")
                        nc.scalar.activation(
                            out=scr,
                            in_=w_t,
                            func=AF.Square,
                            accum_out=sumsq[:, tt : tt + 1],
                        )
                return w_sb, sumsq

            def stage_fin(b, w_sb, sumsq):
                """sqrt (ACT, emitted adjacently for pairs of batches to halve
                activation-table swaps), GpSimd normalize, store bf16."""
                rsq = small.tile([P, NT], F32, tag="rsq", bufs=4)
                nc.scalar.activation(out=rsq, in_=sumsq, func=AF.Sqrt, bias=eps_t)
                ob = outp.tile([P, NT, S], BF16, tag="ob")
                for tt in range(NT):
                    nc.gpsimd.normalize_recip(
                        out_ap=ob[:, tt, :],
                        in_ap=w_sb[:, tt, :],
                        denom_ap=rsq[:, tt : tt + 1],
                    )
                nc.scalar.dma_start(out=out.ap()[b], in_=ob)

            def stage_fin_last(b, w_sb, sumsq):
                """Drain finalize: normalizes split Pool/DVE, two half-DMAs."""
                rsq = small.tile([P, NT], F32, tag="rsq", bufs=4)
                nc.scalar.activation(out=rsq, in_=sumsq, func=AF.Sqrt, bias=eps_t)
                vv = small.tile([P, NT], F32, tag="vv")
                nc.vector.reciprocal_approx_fast(out=vv, in_=rsq)
                ob = outp.tile([P, NT, S], BF16, tag="ob")
                for tt in range(NT):
                    if tt % 2 == 0:
                        nc.gpsimd.normalize_recip(
                            out_ap=ob[:, tt, :],
                            in_ap=w_sb[:, tt, :],
                            denom_ap=rsq[:, tt : tt + 1],
                        )
                    else:
                        nc.vector.tensor_scalar_mul(
                            ob[:, tt, :], w_sb[:, tt, :], vv[:, tt : tt + 1]
                        )
                    if tt == 1:
                        nc.scalar.dma_start(
                            out=out.ap()[b, :, 0:2, :], in_=ob[:, 0:2, :]
                        )
                nc.scalar.dma_start(out=out.ap()[b, :, 2:4, :], in_=ob[:, 2:4, :])

            def dma_x(b):
                t1 = xin.tile([P, KT1, S], F32R, tag="x1", bufs=3)
                nc.sync.dma_start(out=t1[:, 0:2, :], in_=x1t.ap()[b, :, 0:2, :])
                nc.sync.dma_start(out=t1[:, 2:4, :], in_=x1t.ap()[b, :, 2:4, :])
                t2 = xin.tile([P, KT2, S], BF16, tag="x2", bufs=3)
                nc.sync.dma_start(out=t2, in_=x2t.ap()[b])
                return t1, t2

            pending = None  # (b, x1_sb, expz) awaiting stages B+C
            fins = []  # (b, w_sb, sumsq) awaiting finalize, flushed in pairs
            x1_cur, x2_cur = x1_first, x2_first
            for i, b in enumerate(batches):
                if i + 1 < len(batches):
                    nxt = dma_x(batches[i + 1])
                else:
                    nxt = (None, None)
                prev = pending
                hold = {}

                def mid_cb():
                    hold["rbc"] = stage_b(prev[0], prev[2])

                expz = stage_a(
                    b, x1_cur, x2_cur, mid_cb if prev is not None else None
                )
                if prev is not None:
                    fins.append(
                        (prev[0],) + stage_c(prev[0], prev[1], prev[2], hold["rbc"])
                    )
                    if len(fins) == 2:
                        for f in fins:
                            stage_fin(*f)
                        fins = []
                pending = (b, x1_cur, expz)
                x1_cur, x2_cur = nxt
            # drain: the last batch's ACT squares precede the shared
            # sqrt-table epoch of the remaining finalizes.
            rbc_last = stage_b(pending[0], pending[2])
            last_c = stage_c(pending[0], pending[1], pending[2], rbc_last)
            for f in fins:
                stage_fin(*f)
            stage_fin_last(pending[0], *last_c)

    nc.compile()
    return nc


def _pack_pmajor(a, nchunks):
    """[.., nchunks*P, S] -> [.., P, nchunks, S] partition-major contiguous."""
    lead = a.shape[:-2]
    a = a.reshape(lead + (nchunks, P, S))
    perm = tuple(range(len(lead))) + (len(lead) + 1, len(lead), len(lead) + 2)
    return np.ascontiguousarray(a.transpose(perm))


_nc_cache = None


def kernel(x1, x2, W_query, W_key, W_value, _trace=False):
    global _nc_cache, last_results
    x1t = _pack_pmajor(
        np.asarray(x1, dtype=np.float32).transpose(0, 2, 1), KT1
    )  # [B, P, KT1, S]
    x2t = _pack_pmajor(
        np.asarray(x2, dtype=np.float32).transpose(0, 2, 1).astype(BFNP), KT2
    )
    wq = _pack_pmajor(np.asarray(W_query, dtype=np.float32), KT1)
    wk = _pack_pmajor(np.asarray(W_key, dtype=np.float32).astype(BFNP), KT2)
    wv = _pack_pmajor(np.asarray(W_value, dtype=np.float32).astype(BFNP), NT)

    if _nc_cache is None:
        _nc_cache = build_nc()
    nc = _nc_cache

    in_maps = []
    for c in range(N_CORES):
        sl = slice(c * BPC, (c + 1) * BPC)
        in_maps.append(
            {"x1t": x1t[sl], "x2t": x2t[sl], "wq": wq, "wk": wk, "wv": wv}
        )
    res = run_bass_kernel_spmd(
        nc, in_maps, core_ids=list(range(N_CORES)), trace=_trace
    )
    last_results = res
    # out: [bpc, P, NT, S] bf16 -> outT [B, S, S] -> untranspose
    outs = [np.asarray(res.results[c]["out"]) for c in range(N_CORES)]
    outT = np.concatenate(outs, axis=0).astype(np.float32)
    outT = outT.transpose(0, 2, 1, 3).reshape(B, S, S)
    return np.ascontiguousarray(outT.transpose(0, 2, 1))


# revision 15
# speedup vs baseline: 1.3631x; 1.0100x over previous
"""Trainium2 Bass kernel for nn_AttentionLayer (B=64, S=512, F=256), 8 cores.

Reference computation (per batch b):
    scores = x1 @ Wq + x2 @ Wk          # [S, S]
    a = softmax(tanh(scores), axis=-1)   # softmax over u
    a2 = a @ Wv                          # [S, S]
    out = a2 * x1                        # elementwise
    out = out * rsqrt(max(sum_s out^2, eps))   # l2-normalize over axis s

Strategy: pure data parallelism -- 8 batches per core, weights replicated.
Everything is computed in a TRANSPOSED layout ([t-or-u partitions, s free]).

v7 design notes (informed by HW traces):
  * x1 is DMA'd ONCE as float32r: stage A streams it as the moving matmul
    operand (1 cycle/row -- same PE rate as bf16), and the epilogue
    bitcasts the same SBUF bytes to f32.  A second bf16 copy of x1 was
    measured to stretch every engine ~20% via SBUF port contention.
  * All weights + x2 are bf16 (halves their DMA; bf16 stationary loads
    avoid the f32r weight-load stretch on stage-A instructions).
  * fp8 DoubleRow measures only 2x per MAC on this HW, so accuracy-safe
    hi+lo digit splits lose to f32r/bf16 -- not used.
  * Stage C consumes the UNNORMALIZED expz; 1/denominator folds into the
    epilogue, so no PE work waits on the rowsum->recip->broadcast chain.
  * Epilogue per t-tile: q = y*x1 and w = q*recip_bc as dtype-pure f32
    tensor_tensor on DVE (mixed-dtype DVE ops hit a ~2.6x slow path);
    sum-of-squares split between ACT Square+accum (Square lives in both
    activation-table sets -- never forces a swap) and DVE stt; final
    per-row 1/sqrt scale on GpSimd normalize_recip (native Q7 op; gpsimd
    tensor_scalar is a ~7.6us/tile software trap -- never use).
  * Sqrt on ACT for batch PAIRS; the drain shares its sqrt-table epoch.
  * Drain finalize: normalizes split Pool/DVE, output in two half-DMAs.
  * All DRAM tensors partition-major; output bf16, upcast on host.
"""

import sys

sys.path.insert(0, "/opt/trn_rl_repo")

import numpy as np
import ml_dtypes

import concourse.bass as bass
import concourse.tile as tile
from concourse import bacc, mybir
from concourse.bass_utils import run_bass_kernel_spmd

B, S, F = 64, 512, 256
N_CORES = 8
BPC = B // N_CORES  # batches per core
P = 128
KT1 = S // P  # 4 k-tiles over t (x1/Wq contraction)
KT2 = F // P  # 2 k-tiles over f (x2/Wk contraction)
NT = S // P  # 4 m-tiles over u (stage A) / t (stage C)
EPS = 1e-12

F32 = mybir.dt.float32
F32R = mybir.dt.float32r
BF16 = mybir.dt.bfloat16
AF = mybir.ActivationFunctionType
ALU = mybir.AluOpType

BFNP = ml_dtypes.bfloat16

last_results = None  # test harness introspection


def build_nc(reps=1, bpc=BPC):
    nc = bacc.Bacc(
        "TRN2", target_bir_lowering=False, debug=False, num_devices=N_CORES
    )
    # Partition-major packed tensors: [.., P, ktiles, S].
    x1t = nc.declare_dram_parameter("x1t", [bpc, P, KT1, S], F32R, isOutput=False)
    x2t = nc.declare_dram_parameter("x2t", [bpc, P, KT2, S], BF16, isOutput=False)
    wq = nc.declare_dram_parameter("wq", [P, KT1, S], F32R, isOutput=False)
    wk = nc.declare_dram_parameter("wk", [P, KT2, S], BF16, isOutput=False)
    wv = nc.declare_dram_parameter("wv", [P, NT, S], BF16, isOutput=False)
    out = nc.declare_dram_parameter("out", [bpc, P, NT, S], BF16, isOutput=True)

    batches = [bb for _ in range(reps) for bb in range(bpc)]

    with tile.TileContext(nc) as tc:
        with (
            tc.tile_pool(name="singles", bufs=1) as singles,
            tc.tile_pool(name="xin", bufs=1) as xin,
            tc.tile_pool(name="work", bufs=2) as work,
            tc.tile_pool(name="small", bufs=2) as small,
            tc.tile_pool(name="outp", bufs=2) as outp,
            tc.tile_pool(name="psA", bufs=2, space="PSUM") as psA,
            tc.tile_pool(name="psY", bufs=3, space="PSUM") as psY,
            tc.tile_pool(name="psR", bufs=1, space="PSUM") as psR,
        ):
            # Startup DMAs in consumption order: the first u-tile needs
            # wq + x1.kt pairs in sequence, then wk + x2.
            b0 = batches[0]
            x1_first = xin.tile([P, KT1, S], F32R, tag="x1", bufs=3)
            wq_t = singles.tile([P, KT1, S], F32R, tag="wq")
            nc.scalar.dma_start(out=wq_t[:, 0:2, :], in_=wq.ap()[:, 0:2, :])
            nc.scalar.dma_start(out=wq_t[:, 2:4, :], in_=wq.ap()[:, 2:4, :])
            nc.sync.dma_start(out=x1_first[:, 0:2, :], in_=x1t.ap()[b0, :, 0:2, :])
            wk_t = singles.tile([P, KT2, S], BF16, tag="wk")
            nc.gpsimd.dma_start(out=wk_t, in_=wk.ap())
            nc.sync.dma_start(out=x1_first[:, 2:4, :], in_=x1t.ap()[b0, :, 2:4, :])
            x2_first = xin.tile([P, KT2, S], BF16, tag="x2", bufs=3)
            nc.sync.dma_start(out=x2_first, in_=x2t.ap()[b0])
            wv_t = singles.tile([P, NT, S], BF16, tag="wv")
            nc.gpsimd.dma_start(out=wv_t, in_=wv.ap())

            ones_col = singles.tile([P, 1], BF16)
            nc.vector.memset(ones_col, 1.0)
            eps_t = singles.tile([P, 1], F32)
            nc.vector.memset(eps_t, EPS)
            warm_sb = singles.tile([P, S], BF16, tag="warm")
            nc.vector.memset(warm_sb, 0.0)

            def pe_warm(n):
                """Dummy matmuls that keep the TensorE p-state high while it
                would otherwise idle (results never read).  The target cycles
                the single psR bank, ordered before the next rowsum by WAW."""
                warm_ps = psR.tile([1, S], F32, tag="rowsum")
                for _ in range(n):
                    nc.tensor.matmul(
                        warm_ps, ones_col, warm_sb, start=True, stop=True
                    )

            # Ramp the PE during the startup DMA window.
            pe_warm(10)

            def stage_a(b, x1_sb, x2_sb, mid_cb=None):
                """scores matmuls in u-tile pairs sharing one 2-bank PSUM
                tile, tanh+exp over pairs.  mid_cb (if set) is emitted
                between the two pair-halves so the previous batch's rowsum
                overlaps this batch's remaining matmuls."""
                expz = work.tile([P, NT, S], BF16, tag="expz", bufs=3)
                for half in range(NT // 2):
                    sc = psA.tile([P, 2, S], F32, tag="scores")
                    for j in range(2):
                        ut = half * 2 + j
                        us = slice(ut * P, (ut + 1) * P)
                        for kt in range(KT1):
                            nc.tensor.matmul(
                                sc[:, j, :],
                                wq_t[:, kt, us],
                                x1_sb[:, kt, :],
                                start=(kt == 0),
                                stop=False,
                            )
                        for kt in range(KT2):
                            nc.tensor.matmul(
                                sc[:, j, :],
                                wk_t[:, kt, us],
                                x2_sb[:, kt, :],
                                start=False,
                                stop=(kt == KT2 - 1),
                            )
                    tanh_t = work.tile([P, 2, S], F32, tag="tanh")
                    nc.scalar.activation(out=tanh_t, in_=sc, func=AF.Tanh)
                    nc.scalar.activation(
                        out=expz[:, half * 2 : half * 2 + 2, :],
                        in_=tanh_t,
                        func=AF.Exp,
                    )
                    if half == 0 and mid_cb is not None:
                        mid_cb()
                return expz

            def stage_b(b, expz):
                """softmax denominator: rowsum matmul -> recip -> f32 bcast."""
                rs = psR.tile([1, S], F32, tag="rowsum")
                for ut in range(NT):
                    nc.tensor.matmul(
                        rs,
                        ones_col,
                        expz[:, ut, :],
                        start=(ut == 0),
                        stop=(ut == NT - 1),
                    )
                recip_f = small.tile([1, S], F32, tag="recipf")
                nc.vector.reciprocal_approx_fast(out=recip_f, in_=rs)
                rbc = small.tile([P, S], F32, tag="rbc")
                nc.gpsimd.partition_broadcast(rbc, recip_f)
                return rbc

            def stage_c(b, x1_sb, expz, rbc):
                """Y matmuls on raw expz; epilogue q=y*x1 -> w=q*rbc (f32 on
                DVE); sum-of-squares split between ACT Square+accum and DVE
                stt to balance engine load."""
                w_sb = outp.tile([P, NT, S], F32, tag="w", bufs=3)
                sumsq = small.tile([P, NT], F32, tag="sumsq", bufs=4)
                for tt in range(NT):
                    y = psY.tile([P, S], F32, tag="y")
                    for ut in range(NT):
                        nc.tensor.matmul(
                            y,
                            wv_t[:, ut, tt * P : (tt + 1) * P],
                            expz[:, ut, :],
                            start=(ut == 0),
                            stop=(ut == NT - 1),
                        )
                    q_t = small.tile([P, S], F32, tag="q")
                    w_t = w_sb[:, tt, :]
                    nc.vector.tensor_tensor(
                        out=q_t, in0=y, in1=x1_sb[:, tt, :].bitcast(F32), op=ALU.mult
                    )
                    nc.vector.tensor_tensor(out=w_t, in0=q_t, in1=rbc, op=ALU.mult)
                    if tt >= 2:
                        scr = small.tile([P, S], F32, tag="scr")
                        nc.vector.scalar_tensor_tensor(
                            out=scr,
                            in0=w_t,
                            scalar=1.0,
                            in1=w_t,
                            op0=ALU.mult,
                            op1=ALU.mult,
                            accum_out=sumsq[:, tt : tt + 1],
                        )
                    else:
                        scr = small.tile([P, S], BF16, tag="scrb")
                        nc.scalar.activation(
                            out=scr,
                            in_=w_t,
                            func=AF.Square,
                            accum_out=sumsq[:, tt : tt + 1],
                        )
                return w_sb, sumsq

            def stage_fin(b, w_sb, sumsq):
                """sqrt (ACT, emitted adjacently for pairs of batches to halve
                activation-table swaps), GpSimd normalize, store bf16."""
                rsq = small.tile([P, NT], F32, tag="rsq", bufs=4)
                nc.scalar.activation(out=rsq, in_=sumsq, func=AF.Sqrt, bias=eps_t)
                ob = outp.tile([P, NT, S], BF16, tag="ob")
                for tt in range(NT):
                    nc.gpsimd.normalize_recip(
                        out_ap=ob[:, tt, :],
                        in_ap=w_sb[:, tt, :],
                        denom_ap=rsq[:, tt : tt + 1],
                    )
                nc.scalar.dma_start(out=out.ap()[b], in_=ob)

            def stage_fin_last(b, w_sb, sumsq):
                """Drain finalize: normalizes split Pool/DVE, two half-DMAs."""
                rsq = small.tile([P, NT], F32, tag="rsq", bufs=4)
                nc.scalar.activation(out=rsq, in_=sumsq, func=AF.Sqrt, bias=eps_t)
                vv = small.tile([P, NT], F32, tag="vv")
                nc.vector.reciprocal_approx_fast(out=vv, in_=rsq)
                ob = outp.tile([P, NT, S], BF16, tag="ob")
                for tt in range(NT):
                    if tt % 2 == 0:
                        nc.gpsimd.normalize_recip(
                            out_ap=ob[:, tt, :],
                            in_ap=w_sb[:, tt, :],
                            denom_ap=rsq[:, tt : tt + 1],
                        )
                    else:
                        nc.vector.tensor_scalar_mul(
                            ob[:, tt, :], w_sb[:, tt, :], vv[:, tt : tt + 1]
                        )
                    if tt == 1:
                        nc.scalar.dma_start(
                            out=out.ap()[b, :, 0:2, :], in_=ob[:, 0:2, :]
                        )
                nc.scalar.dma_start(out=out.ap()[b, :, 2:4, :], in_=ob[:, 2:4, :])

            def dma_x(b):
                t1 = xin.tile([P, KT1, S], F32R, tag="x1", bufs=3)
                nc.sync.dma_start(out=t1[:, 0:2, :], in_=x1t.ap()[b, :, 0:2, :])
                nc.sync.dma_start(out=t1[:, 2:4, :], in_=x1t.ap()[b, :, 2:4, :])
                t2 = xin.tile([P, KT2, S], BF16, tag="x2", bufs=3)
                nc.sync.dma_start(out=t2, in_=x2t.ap()[b])
                return t1, t2

            pending = None  # (b, x1_sb, expz) awaiting stages B+C
            fins = []  # (b, w_sb, sumsq) awaiting finalize, flushed in pairs
            x1_cur, x2_cur = x1_first, x2_first
            for i, b in enumerate(batches):
                if i + 1 < len(batches):
                    nxt = dma_x(batches[i + 1])
                else:
                    nxt = (None, None)
                prev = pending
                hold = {}

                def mid_cb():
                    hold["rbc"] = stage_b(prev[0], prev[2])

                expz = stage_a(
                    b, x1_cur, x2_cur, mid_cb if prev is not None else None
                )
                if prev is not None:
                    fins.append(
                        (prev[0],) + stage_c(prev[0], prev[1], prev[2], hold["rbc"])
                    )
                    if len(fins) == 2:
                        for f in fins:
                            stage_fin(*f)
                        fins = []
                pending = (b, x1_cur, expz)
                x1_cur, x2_cur = nxt
            # drain: dummy matmuls keep the PE clock hot while the last
            # batch's exp completes; the last batch's ACT squares precede
            # the shared sqrt-table epoch of the remaining finalizes.
            pe_warm(6)
            rbc_last = stage_b(pending[0], pending[2])
            last_c = stage_c(pending[0], pending[1], pending[2], rbc_last)
            for f in fins:
                stage_fin(*f)
            stage_fin_last(pending[0], *last_c)

    nc.compile()
    return nc


def _pack_pmajor(a, nchunks):
    """[.., nchunks*P, S] -> [.., P, nchunks, S] partition-major contiguous."""
    lead = a.shape[:-2]
    a = a.reshape(lead + (nchunks, P, S))
    perm = tuple(range(len(lead))) + (len(lead) + 1, len(lead), len(lead) + 2)
    return np.ascontiguousarray(a.transpose(perm))


_nc_cache = None


def kernel(x1, x2, W_query, W_key, W_value, _trace=False):
    global _nc_cache, last_results
    x1t = _pack_pmajor(
        np.asarray(x1, dtype=np.float32).transpose(0, 2, 1), KT1
    )  # [B, P, KT1, S]
    x2t = _pack_pmajor(
        np.asarray(x2, dtype=np.float32).transpose(0, 2, 1).astype(BFNP), KT2
    )
    wq = _pack_pmajor(np.asarray(W_query, dtype=np.float32), KT1)
    wk = _pack_pmajor(np.asarray(W_key, dtype=np.float32).astype(BFNP), KT2)
    wv = _pack_pmajor(np.asarray(W_value, dtype=np.float32).astype(BFNP), NT)

    if _nc_cache is None:
        _nc_cache = build_nc()
    nc = _nc_cache

    in_maps = []
    for c in range(N_CORES):
        sl = slice(c * BPC, (c + 1) * BPC)
        in_maps.append(
            {"x1t": x1t[sl], "x2t": x2t[sl], "wq": wq, "wk": wk, "wv": wv}
        )
    res = run_bass_kernel_spmd(
        nc, in_maps, core_ids=list(range(N_CORES)), trace=_trace
    )
    last_results = res
    # out: [bpc, P, NT, S] bf16 -> outT [B, S, S] -> untranspose
    outs = [np.asarray(res.results[c]["out"]) for c in range(N_CORES)]
    outT = np.concatenate(outs, axis=0).astype(np.float32)
    outT = outT.transpose(0, 2, 1, 3).reshape(B, S, S)
    return np.ascontiguousarray(outT.transpose(0, 2, 1))
